# revision 15
# baseline (speedup 1.0000x reference)
"""MoRALinear fused kernel for 8x TRN2 NeuronCores — fp8/bf16 hybrid.

Math: reference computes
    y = x @ W.T + b + tile(lora_A(chunk_sum(x)))
Data-parallel over tokens across 8 cores (weights replicated).

Per-core GEMM [2048, 4096] x [4096, 4096] split three ways:
  - K32=1536 k-columns in bf16 (1.0 PE cycles/row)       — negligible error
  - K8=2560 k-columns in fp8 e4m3 DoubleRow (2x rate)    — err ~0.019 of scale
  - MoRA adapter as a separate bf16 GEMM on the host-precomputed
    chunk-sum in_x = sum of 4 k-chunks of x ([2048,1024] @ A.T),
    computed once and reused for all 4 tiled output replicas.
All three accumulate into shared PSUM banks.  The fp8 products carry an
SX*SW scale (x*SX, W*SW; the pair is error-search-tuned); the bf16
weights are pre-scaled by SX*SW on host so PSUM is uniformly scaled;
eviction divides by SX*SW and adds the bias, emitting bf16 outputs.

Schedule per block of 1024 tokens: adapter GEMM (fills the DMA cold
start), then 8 n-tiles of 512 outputs; per (m-strip, n-tile) bank:
12 bf16 matmuls + 20 fp8 DoubleRow matmuls (no bf16<->fp8 dtype-switch
penalty), 3-op DVE eviction.  Measured: ~689 us, rel err 0.0189 (gate 2e-2).
"""

import numpy as np
import ml_dtypes

import concourse.bass as bass
import concourse.mybir as mybir
import concourse.tile as tile
from concourse import bacc
from concourse.bass_utils import run_bass_kernel_spmd

B, S, IN_F, OUT_F, R = 4, 4096, 4096, 4096, 1024
N_CORES = 8
TOKENS = B * S                  # 16384
M_PER_CORE = TOKENS // N_CORES  # 2048

P = 128
MB = 1024                       # tokens per block
N_BLOCKS = M_PER_CORE // MB     # 2
MT = MB // P                    # 8 m-strips per block
NTILE = 512
NT = OUT_F // NTILE             # 8 n-tiles

K8 = 2560                       # fp8 k-columns (last K8 of IN_F)
K32 = IN_F - K8                 # bf16 k-columns
KS32 = K32 // P                 # 12 bf16 k-strips
PR8 = K8 // 256                 # 10 fp8 k-pairs
KA = R // P                     # 8 adapter k-strips
AH = R // NTILE                 # 2 adapter n-halves

SX, SW = 39.0, 1400.0           # fp8 scales (chosen by error search)
DEQ = 1.0 / (SX * SW)

F32 = mybir.dt.float32
BF16 = mybir.dt.bfloat16
FP8 = mybir.dt.float8e4
E8 = ml_dtypes.float8_e4m3
DR = mybir.MatmulPerfMode.DoubleRow

W32_CHUNK = 6                   # bf16 k-strips per W DMA (0.75 MB)
W8R = 4 * PR8                   # w8 rows per n-tile


def build_nc():
    nc = bacc.Bacc("TRN2", target_bir_lowering=False, debug=False)

    with tile.TileContext(nc) as tc:
        with tc.tile_pool(name="dram", bufs=1, space="DRAM") as dram:
            # x32t[p, blk*KS32+ks, m] = x[blk*MB+m, ks*128+p]         (bf16)
            x32d = dram.tile([P, N_BLOCKS * KS32, MB], BF16,
                             kind="ExternalInput", name="x32", uniquify=False)
            # x8d[p, (blk*PR8+pr)*2+i, m] = e4m3(32*x[blk*MB+m, K32+pr*256+i*128+p])
            x8d = dram.tile([P, N_BLOCKS * PR8 * 2, MB], FP8,
                            kind="ExternalInput", name="x8", uniquify=False)
            # w32d[p, nt*KS32+ks, j] = bf16(2^15 * W[nt*512+j, ks*128+p])
            w32d = dram.tile([P, NT * KS32, NTILE], BF16,
                             kind="ExternalInput", name="w32", uniquify=False)
            # w8d[p, nt*W8R + h*(2*PR8) + pr*2 + i, j] =
            #     e4m3(1024 * W[nt*512+h*256+j, K32+pr*256+i*128+p])
            w8d = dram.tile([P, NT * W8R, 256], FP8,
                            kind="ExternalInput", name="w8", uniquify=False)
            # inxd[p, blk*KA+ka, m] = bf16(in_x[blk*MB+m, ka*128+p])
            inxd = dram.tile([P, N_BLOCKS * KA, MB], BF16,
                             kind="ExternalInput", name="inx", uniquify=False)
            # wad[p, ka, a] = bf16(2^15 * A[a, ka*128+p])
            wad = dram.tile([P, KA, R], BF16,
                            kind="ExternalInput", name="wa", uniquify=False)
            bias_d = dram.tile([P, OUT_F], BF16,
                               kind="ExternalInput", name="bias", uniquify=False)
            out_d = dram.tile([P, N_BLOCKS * MT, OUT_F], BF16,
                              kind="ExternalOutput", name="out", uniquify=False)

        with (
            tc.tile_pool(name="const", bufs=1) as const,
            tc.tile_pool(name="w32pool", bufs=4) as w32pool,
            tc.tile_pool(name="w8pool", bufs=2) as w8pool,
            tc.tile_pool(name="tpool", bufs=3) as tpool,
            tc.tile_pool(name="opool", bufs=4) as opool,
            tc.tile_pool(name="pspool", bufs=8, space="PSUM") as pspool,
        ):
            # resident tensors
            x32_sb = const.tile([P, N_BLOCKS * KS32, MB], BF16, name="x32_sb")
            x8_sb = const.tile([P, N_BLOCKS * PR8 * 2, MB], FP8, name="x8_sb")
            inx_tiles = [const.tile([P, MB], BF16, name=f"inx{k}") for k in range(KA)]
            wa_tiles = [const.tile([P, R], BF16, name=f"wa{k}") for k in range(KA)]
            adapt_sb = const.tile([P, MT, R], BF16, name="adapt_sb")  # per block
            bias_sb = const.tile([P, OUT_F], BF16, name="bias_sb")

            n_w32_chunks = KS32 // W32_CHUNK

            def load_w32(nt):
                tiles = []
                for c in range(n_w32_chunks):
                    wk = w32pool.tile([P, W32_CHUNK, NTILE], BF16, name="w32k")
                    row = nt * KS32 + c * W32_CHUNK
                    nc.sync.dma_start(wk[:], w32d[:, row:row + W32_CHUNK, :])
                    tiles.append(wk)
                return tiles

            def load_w8(nt):
                w8t = w8pool.tile([P, W8R, 256], FP8, name="w8t")
                nc.sync.dma_start(w8t[:], w8d[:, nt * W8R:(nt + 1) * W8R, :])
                return w8t

            # ---- cold start: adapter inputs first (per-strip tiles so the
            # adapter GEMM starts ~3us in), then block-0 weights/x ----
            for ka in range(KA):
                nc.sync.dma_start(wa_tiles[ka][:], wad[:, ka, :])
                nc.sync.dma_start(inx_tiles[ka][:], inxd[:, ka, :])
            w32_cur = load_w32(0)
            # block-0 x loads (consumed ~27us in, after the adapter GEMM)
            nc.sync.dma_start(
                x32_sb[:, 0:KS32, :], x32d[:, 0:KS32, :])
            w8_cur = load_w8(0)
            nc.sync.dma_start(
                x8_sb[:, 0:PR8 * 2, :], x8d[:, 0:PR8 * 2, :])
            nc.sync.dma_start(bias_sb[:], bias_d[:])

            for blk in range(N_BLOCKS):
                xrow32 = blk * KS32
                xrow8 = blk * PR8 * 2

                # ---- phase A: adapter GEMM into adapt_sb (SX*SW-scaled) ----
                for ah in range(AH):
                    for ms in range(MT):
                        psa = pspool.tile([P, NTILE], F32, name="ps")
                        for ka in range(KA):
                            nc.tensor.matmul(
                                psa[:],
                                lhsT=inx_tiles[ka][:, ms * P:(ms + 1) * P],
                                rhs=wa_tiles[ka][:, ah * NTILE:(ah + 1) * NTILE],
                                start=(ka == 0), stop=(ka == KA - 1),
                            )
                        nc.vector.tensor_scalar_mul(
                            adapt_sb[:, ms, ah * NTILE:(ah + 1) * NTILE],
                            psa[:], 1.0)

                # prefetch next block's adapter inputs right after last read
                if blk + 1 < N_BLOCKS:
                    for ka in range(KA):
                        nc.sync.dma_start(
                            inx_tiles[ka][:], inxd[:, (blk + 1) * KA + ka, :])

                # ---- phase B: n-tiles ----
                for nt in range(NT):
                    w32_nt, w8_nt = w32_cur, w8_cur
                    # prefetch next tile's weights
                    nxt = nt + 1
                    if nxt < NT:
                        w32_cur = load_w32(nxt)
                        w8_cur = load_w8(nxt)
                    elif blk + 1 < N_BLOCKS:
                        w32_cur = load_w32(0)
                        w8_cur = load_w8(0)
                        # next block's x after this block's last-tile weights
                        nc.sync.dma_start(
                            x32_sb[:, KS32:2 * KS32, :],
                            x32d[:, KS32:2 * KS32, :])
                        nc.sync.dma_start(
                            x8_sb[:, PR8 * 2:2 * PR8 * 2, :],
                            x8d[:, PR8 * 2:2 * PR8 * 2, :])

                    ns = slice(nt * NTILE, (nt + 1) * NTILE)
                    amod = (nt * NTILE) % R

                    for ms in range(MT):
                        ps = pspool.tile([P, NTILE], F32, name="ps")
                        msl = slice(ms * P, (ms + 1) * P)
                        # bf16 phase
                        for ks in range(KS32):
                            nc.tensor.matmul(
                                ps[:],
                                lhsT=x32_sb[:, xrow32 + ks, msl],
                                rhs=w32_nt[ks // W32_CHUNK][:, ks % W32_CHUNK, :],
                                start=(ks == 0), stop=False,
                            )
                        # fp8 phase
                        for h in range(2):
                            for pr in range(PR8):
                                nc.tensor.matmul(
                                    ps[:, h * 256:(h + 1) * 256],
                                    lhsT=x8_sb[:, xrow8 + pr * 2:xrow8 + pr * 2 + 2, msl],
                                    rhs=w8_nt[:, h * 2 * PR8 + pr * 2:h * 2 * PR8 + pr * 2 + 2, :],
                                    start=False, stop=(pr == PR8 - 1),
                                    perf_mode=DR,
                                )
                        # eviction: (+adapter, *2^-15, +bias) -> bf16
                        t1 = tpool.tile([P, NTILE], F32, name="t1")
                        nc.vector.tensor_add(
                            out=t1[:], in0=ps[:],
                            in1=adapt_sb[:, ms, amod:amod + NTILE])
                        nc.vector.tensor_scalar_mul(t1[:], t1[:], DEQ)
                        ot = opool.tile([P, NTILE], BF16, name="ot")
                        nc.vector.tensor_add(
                            out=ot[:], in0=t1[:], in1=bias_sb[:, ns])
                        nc.sync.dma_start(
                            out_d[:, blk * MT + ms, ns], ot[:])

    nc.compile()
    return nc


def prep_inputs(x, W, b, A, n_cores: int = N_CORES):
    """Host-side shard + layout prep. Returns in_maps for run_bass_kernel_spmd."""
    x = np.asarray(x, dtype=np.float32)
    W = np.asarray(W, dtype=np.float32)
    b = np.asarray(b, dtype=np.float32)
    A = np.asarray(A, dtype=np.float32)

    # shared weights
    W32 = W[:, :K32]
    w32 = np.ascontiguousarray(
        (W32 * (SX * SW)).reshape(NT, NTILE, KS32, P)
        .transpose(3, 0, 2, 1).reshape(P, NT * KS32, NTILE)
    ).astype(ml_dtypes.bfloat16)
    W8 = W[:, K32:]
    w8 = np.ascontiguousarray(
        np.asarray(W8 * SW, dtype=E8)
        .reshape(NT, 2, 256, PR8, 2, P)
        .transpose(5, 0, 1, 3, 4, 2).reshape(P, NT * W8R, 256)
    )
    wa = np.ascontiguousarray(
        (A * (SX * SW)).reshape(R, KA, P).transpose(2, 1, 0)
    ).astype(ml_dtypes.bfloat16)
    bias = np.ascontiguousarray(
        np.broadcast_to(b, (P, OUT_F))).astype(ml_dtypes.bfloat16)

    x_flat = x.reshape(TOKENS, IN_F)
    in_maps = []
    for c in range(n_cores):
        shard = x_flat[c * M_PER_CORE:(c + 1) * M_PER_CORE]
        x32t = np.ascontiguousarray(
            shard[:, :K32].reshape(N_BLOCKS, MB, KS32, P)
            .transpose(3, 0, 2, 1).reshape(P, N_BLOCKS * KS32, MB)
        ).astype(ml_dtypes.bfloat16)
        x8t = np.ascontiguousarray(
            np.asarray(shard[:, K32:] * SX, dtype=E8)
            .reshape(N_BLOCKS, MB, PR8, 2, P)
            .transpose(4, 0, 2, 3, 1).reshape(P, N_BLOCKS * PR8 * 2, MB)
        )
        in_x = shard.reshape(M_PER_CORE, IN_F // R, R).sum(axis=1)
        inxt = np.ascontiguousarray(
            in_x.reshape(N_BLOCKS, MB, KA, P)
            .transpose(3, 0, 2, 1).reshape(P, N_BLOCKS * KA, MB)
        ).astype(ml_dtypes.bfloat16)
        in_maps.append({
            "x32": x32t, "x8": x8t, "w32": w32, "w8": w8,
            "inx": inxt, "wa": wa, "bias": bias,
        })
    return in_maps


def unshard(results):
    shards = []
    for res in results:
        o = np.asarray(res["out"]).astype(np.float32)  # [P, MT*blocks, OUT_F]
        shards.append(o.transpose(1, 0, 2).reshape(M_PER_CORE, OUT_F))
    return np.concatenate(shards, axis=0).reshape(B, S, OUT_F)


_NC_CACHE = {}


def run(x, W, b, A, trace=False, tmpdir=None, **spmd_kwargs):
    if "nc" not in _NC_CACHE:
        _NC_CACHE["nc"] = build_nc()
    nc = _NC_CACHE["nc"]
    in_maps = prep_inputs(x, W, b, A)
    br = run_bass_kernel_spmd(
        nc, in_maps, list(range(N_CORES)), trace=trace, tmpdir=tmpdir, **spmd_kwargs
    )
    return unshard(br.results), br


def kernel(x, W, b, A):
    last_err = None
    for attempt in range(3):
        try:
            out, _ = run(x, W, b, A)
            return out.astype(np.float32)
        except Exception as e:  # transient device flakes (e.g. NRT exec errors)
            last_err = e
            _NC_CACHE.clear()
            import time

            time.sleep(5)
    raise last_err


# revision 16
# speedup vs baseline: 1.0012x; 1.0012x over previous
"""MoRALinear fused kernel for 8x TRN2 NeuronCores — fp8/bf16 hybrid.

Math: reference computes
    y = x @ W.T + b + tile(lora_A(chunk_sum(x)))
Data-parallel over tokens across 8 cores (weights replicated).

Per-core GEMM [2048, 4096] x [4096, 4096] split three ways:
  - K32=1536 k-columns in bf16 (1.0 PE cycles/row)       — negligible error
  - K8=2560 k-columns in fp8 e4m3 DoubleRow (2x rate)    — err ~0.019 of scale
  - MoRA adapter as a separate bf16 GEMM on the host-precomputed
    chunk-sum in_x = sum of 4 k-chunks of x ([2048,1024] @ A.T),
    computed once and reused for all 4 tiled output replicas.
All three accumulate into shared PSUM banks.  The fp8 products carry an
SX*SW scale (x*SX, W*SW; the pair is error-search-tuned); the bf16
weights are pre-scaled by SX*SW on host so PSUM is uniformly scaled;
eviction divides by SX*SW and adds the bias, emitting bf16 outputs.

Schedule per block of 1024 tokens: adapter GEMM (fills the DMA cold
start), then 8 n-tiles of 512 outputs; per (m-strip, n-tile) bank:
12 bf16 matmuls + 20 fp8 DoubleRow matmuls (no bf16<->fp8 dtype-switch
penalty), 3-op DVE eviction.  Measured: ~689 us, rel err 0.0189 (gate 2e-2).
"""

import numpy as np
import ml_dtypes

import concourse.bass as bass
import concourse.mybir as mybir
import concourse.tile as tile
from concourse import bacc
from concourse.bass_utils import run_bass_kernel_spmd

B, S, IN_F, OUT_F, R = 4, 4096, 4096, 4096, 1024
N_CORES = 8
TOKENS = B * S                  # 16384
M_PER_CORE = TOKENS // N_CORES  # 2048

P = 128
MB = 1024                       # tokens per block
N_BLOCKS = M_PER_CORE // MB     # 2
MT = MB // P                    # 8 m-strips per block
NTILE = 512
NT = OUT_F // NTILE             # 8 n-tiles

K8 = 2560                       # fp8 k-columns (last K8 of IN_F)
K32 = IN_F - K8                 # bf16 k-columns
KS32 = K32 // P                 # 12 bf16 k-strips
PR8 = K8 // 256                 # 10 fp8 k-pairs
KA = R // P                     # 8 adapter k-strips
AH = R // NTILE                 # 2 adapter n-halves

SX, SW = 36.0, 1260.0           # fp8 scales (chosen by error search)
DEQ = 1.0 / (SX * SW)

F32 = mybir.dt.float32
BF16 = mybir.dt.bfloat16
FP8 = mybir.dt.float8e4
E8 = ml_dtypes.float8_e4m3
DR = mybir.MatmulPerfMode.DoubleRow

W32_CHUNK = 6                   # bf16 k-strips per W DMA (0.75 MB)
W8R = 4 * PR8                   # w8 rows per n-tile


def build_nc():
    nc = bacc.Bacc("TRN2", target_bir_lowering=False, debug=False)

    with tile.TileContext(nc) as tc:
        with tc.tile_pool(name="dram", bufs=1, space="DRAM") as dram:
            # x32t[p, blk*KS32+ks, m] = x[blk*MB+m, ks*128+p]         (bf16)
            x32d = dram.tile([P, N_BLOCKS * KS32, MB], BF16,
                             kind="ExternalInput", name="x32", uniquify=False)
            # x8d[p, (blk*PR8+pr)*2+i, m] = e4m3(32*x[blk*MB+m, K32+pr*256+i*128+p])
            x8d = dram.tile([P, N_BLOCKS * PR8 * 2, MB], FP8,
                            kind="ExternalInput", name="x8", uniquify=False)
            # w32d[p, nt*KS32+ks, j] = bf16(2^15 * W[nt*512+j, ks*128+p])
            w32d = dram.tile([P, NT * KS32, NTILE], BF16,
                             kind="ExternalInput", name="w32", uniquify=False)
            # w8d[p, nt*W8R + h*(2*PR8) + pr*2 + i, j] =
            #     e4m3(1024 * W[nt*512+h*256+j, K32+pr*256+i*128+p])
            w8d = dram.tile([P, NT * W8R, 256], FP8,
                            kind="ExternalInput", name="w8", uniquify=False)
            # inxd[p, blk*KA+ka, m] = bf16(in_x[blk*MB+m, ka*128+p])
            inxd = dram.tile([P, N_BLOCKS * KA, MB], BF16,
                             kind="ExternalInput", name="inx", uniquify=False)
            # wad[p, ka, a] = bf16(2^15 * A[a, ka*128+p])
            wad = dram.tile([P, KA, R], BF16,
                            kind="ExternalInput", name="wa", uniquify=False)
            bias_d = dram.tile([P, OUT_F], BF16,
                               kind="ExternalInput", name="bias", uniquify=False)
            out_d = dram.tile([P, N_BLOCKS * MT, OUT_F], BF16,
                              kind="ExternalOutput", name="out", uniquify=False)

        with (
            tc.tile_pool(name="const", bufs=1) as const,
            tc.tile_pool(name="w32pool", bufs=4) as w32pool,
            tc.tile_pool(name="w8pool", bufs=2) as w8pool,
            tc.tile_pool(name="tpool", bufs=3) as tpool,
            tc.tile_pool(name="opool", bufs=4) as opool,
            tc.tile_pool(name="pspool", bufs=8, space="PSUM") as pspool,
        ):
            # resident tensors
            x32_sb = const.tile([P, N_BLOCKS * KS32, MB], BF16, name="x32_sb")
            x8_sb = const.tile([P, N_BLOCKS * PR8 * 2, MB], FP8, name="x8_sb")
            inx_tiles = [const.tile([P, MB], BF16, name=f"inx{k}") for k in range(KA)]
            wa_tiles = [const.tile([P, R], BF16, name=f"wa{k}") for k in range(KA)]
            adapt_sb = const.tile([P, MT, R], BF16, name="adapt_sb")  # per block
            bias_sb = const.tile([P, OUT_F], BF16, name="bias_sb")

            n_w32_chunks = KS32 // W32_CHUNK

            def load_w32(nt):
                tiles = []
                for c in range(n_w32_chunks):
                    wk = w32pool.tile([P, W32_CHUNK, NTILE], BF16, name="w32k")
                    row = nt * KS32 + c * W32_CHUNK
                    nc.sync.dma_start(wk[:], w32d[:, row:row + W32_CHUNK, :])
                    tiles.append(wk)
                return tiles

            def load_w8(nt):
                w8t = w8pool.tile([P, W8R, 256], FP8, name="w8t")
                nc.sync.dma_start(w8t[:], w8d[:, nt * W8R:(nt + 1) * W8R, :])
                return w8t

            # ---- cold start: adapter inputs first (per-strip tiles so the
            # adapter GEMM starts ~3us in), then block-0 weights/x ----
            for ka in range(KA):
                nc.sync.dma_start(wa_tiles[ka][:], wad[:, ka, :])
                nc.sync.dma_start(inx_tiles[ka][:], inxd[:, ka, :])
            w32_cur = load_w32(0)
            # block-0 x loads (consumed ~27us in, after the adapter GEMM)
            nc.sync.dma_start(
                x32_sb[:, 0:KS32, :], x32d[:, 0:KS32, :])
            w8_cur = load_w8(0)
            nc.sync.dma_start(
                x8_sb[:, 0:PR8 * 2, :], x8d[:, 0:PR8 * 2, :])
            nc.sync.dma_start(bias_sb[:], bias_d[:])

            for blk in range(N_BLOCKS):
                xrow32 = blk * KS32
                xrow8 = blk * PR8 * 2

                # ---- phase A: adapter GEMM into adapt_sb (SX*SW-scaled) ----
                for ah in range(AH):
                    for ms in range(MT):
                        psa = pspool.tile([P, NTILE], F32, name="ps")
                        for ka in range(KA):
                            nc.tensor.matmul(
                                psa[:],
                                lhsT=inx_tiles[ka][:, ms * P:(ms + 1) * P],
                                rhs=wa_tiles[ka][:, ah * NTILE:(ah + 1) * NTILE],
                                start=(ka == 0), stop=(ka == KA - 1),
                            )
                        nc.vector.tensor_scalar_mul(
                            adapt_sb[:, ms, ah * NTILE:(ah + 1) * NTILE],
                            psa[:], 1.0)

                # prefetch next block's adapter inputs right after last read
                if blk + 1 < N_BLOCKS:
                    for ka in range(KA):
                        nc.sync.dma_start(
                            inx_tiles[ka][:], inxd[:, (blk + 1) * KA + ka, :])

                # ---- phase B: n-tiles ----
                for nt in range(NT):
                    w32_nt, w8_nt = w32_cur, w8_cur
                    # prefetch next tile's weights
                    nxt = nt + 1
                    if nxt < NT:
                        w32_cur = load_w32(nxt)
                        w8_cur = load_w8(nxt)
                    elif blk + 1 < N_BLOCKS:
                        w32_cur = load_w32(0)
                        w8_cur = load_w8(0)
                        # next block's x after this block's last-tile weights
                        nc.sync.dma_start(
                            x32_sb[:, KS32:2 * KS32, :],
                            x32d[:, KS32:2 * KS32, :])
                        nc.sync.dma_start(
                            x8_sb[:, PR8 * 2:2 * PR8 * 2, :],
                            x8d[:, PR8 * 2:2 * PR8 * 2, :])

                    ns = slice(nt * NTILE, (nt + 1) * NTILE)
                    amod = (nt * NTILE) % R

                    for ms in range(MT):
                        ps = pspool.tile([P, NTILE], F32, name="ps")
                        msl = slice(ms * P, (ms + 1) * P)
                        # bf16 phase
                        for ks in range(KS32):
                            nc.tensor.matmul(
                                ps[:],
                                lhsT=x32_sb[:, xrow32 + ks, msl],
                                rhs=w32_nt[ks // W32_CHUNK][:, ks % W32_CHUNK, :],
                                start=(ks == 0), stop=False,
                            )
                        # fp8 phase
                        for h in range(2):
                            for pr in range(PR8):
                                nc.tensor.matmul(
                                    ps[:, h * 256:(h + 1) * 256],
                                    lhsT=x8_sb[:, xrow8 + pr * 2:xrow8 + pr * 2 + 2, msl],
                                    rhs=w8_nt[:, h * 2 * PR8 + pr * 2:h * 2 * PR8 + pr * 2 + 2, :],
                                    start=False, stop=(pr == PR8 - 1),
                                    perf_mode=DR,
                                )
                        # eviction: (+adapter, *2^-15, +bias) -> bf16
                        t1 = tpool.tile([P, NTILE], F32, name="t1")
                        nc.vector.tensor_add(
                            out=t1[:], in0=ps[:],
                            in1=adapt_sb[:, ms, amod:amod + NTILE])
                        nc.vector.tensor_scalar_mul(t1[:], t1[:], DEQ)
                        ot = opool.tile([P, NTILE], BF16, name="ot")
                        nc.vector.tensor_add(
                            out=ot[:], in0=t1[:], in1=bias_sb[:, ns])
                        nc.sync.dma_start(
                            out_d[:, blk * MT + ms, ns], ot[:])

    nc.compile()
    return nc


def prep_inputs(x, W, b, A, n_cores: int = N_CORES):
    """Host-side shard + layout prep. Returns in_maps for run_bass_kernel_spmd."""
    x = np.asarray(x, dtype=np.float32)
    W = np.asarray(W, dtype=np.float32)
    b = np.asarray(b, dtype=np.float32)
    A = np.asarray(A, dtype=np.float32)

    # shared weights
    W32 = W[:, :K32]
    w32 = np.ascontiguousarray(
        (W32 * (SX * SW)).reshape(NT, NTILE, KS32, P)
        .transpose(3, 0, 2, 1).reshape(P, NT * KS32, NTILE)
    ).astype(ml_dtypes.bfloat16)
    W8 = W[:, K32:]
    w8 = np.ascontiguousarray(
        np.asarray(W8 * SW, dtype=E8)
        .reshape(NT, 2, 256, PR8, 2, P)
        .transpose(5, 0, 1, 3, 4, 2).reshape(P, NT * W8R, 256)
    )
    wa = np.ascontiguousarray(
        (A * (SX * SW)).reshape(R, KA, P).transpose(2, 1, 0)
    ).astype(ml_dtypes.bfloat16)
    bias = np.ascontiguousarray(
        np.broadcast_to(b, (P, OUT_F))).astype(ml_dtypes.bfloat16)

    x_flat = x.reshape(TOKENS, IN_F)
    in_maps = []
    for c in range(n_cores):
        shard = x_flat[c * M_PER_CORE:(c + 1) * M_PER_CORE]
        x32t = np.ascontiguousarray(
            shard[:, :K32].reshape(N_BLOCKS, MB, KS32, P)
            .transpose(3, 0, 2, 1).reshape(P, N_BLOCKS * KS32, MB)
        ).astype(ml_dtypes.bfloat16)
        x8t = np.ascontiguousarray(
            np.asarray(shard[:, K32:] * SX, dtype=E8)
            .reshape(N_BLOCKS, MB, PR8, 2, P)
            .transpose(4, 0, 2, 3, 1).reshape(P, N_BLOCKS * PR8 * 2, MB)
        )
        in_x = shard.reshape(M_PER_CORE, IN_F // R, R).sum(axis=1)
        inxt = np.ascontiguousarray(
            in_x.reshape(N_BLOCKS, MB, KA, P)
            .transpose(3, 0, 2, 1).reshape(P, N_BLOCKS * KA, MB)
        ).astype(ml_dtypes.bfloat16)
        in_maps.append({
            "x32": x32t, "x8": x8t, "w32": w32, "w8": w8,
            "inx": inxt, "wa": wa, "bias": bias,
        })
    return in_maps


def unshard(results):
    shards = []
    for res in results:
        o = np.asarray(res["out"]).astype(np.float32)  # [P, MT*blocks, OUT_F]
        shards.append(o.transpose(1, 0, 2).reshape(M_PER_CORE, OUT_F))
    return np.concatenate(shards, axis=0).reshape(B, S, OUT_F)


_NC_CACHE = {}


def run(x, W, b, A, trace=False, tmpdir=None, **spmd_kwargs):
    if "nc" not in _NC_CACHE:
        _NC_CACHE["nc"] = build_nc()
    nc = _NC_CACHE["nc"]
    in_maps = prep_inputs(x, W, b, A)
    br = run_bass_kernel_spmd(
        nc, in_maps, list(range(N_CORES)), trace=trace, tmpdir=tmpdir, **spmd_kwargs
    )
    return unshard(br.results), br


def kernel(x, W, b, A):
    last_err = None
    for attempt in range(3):
        try:
            out, _ = run(x, W, b, A)
            return out.astype(np.float32)
        except Exception as e:  # transient device flakes (e.g. NRT exec errors)
            last_err = e
            _NC_CACHE.clear()
            import time

            time.sleep(5)
    raise last_err
